# revision 21
# baseline (speedup 1.0000x reference)
"""Trainium2 Bass kernel for nn_FChCombxValEncoder (HDC n-gram encoder).

Computation: idx = quantize(x) -> signal = signals_weight[idx] -> bind with
feat_weight -> 4-gram product with per-step D-rolls -> bundle sum -> sign.

Distribution (v2, D-shard): the hypervector dimension D=10000 is sharded
across the 8 cores -- core m owns output columns [1250m, 1250m+1250).  Each
core sees ALL 4096 feature rows but only a 1255-column slice (with mod-D
wrap) of the level table and feat weights, so the whole n-gram bundle sum
for its slice is local: NO collective, no orphan rows, no cross-core
reduce.  The host concatenates the 8 slices and applies the final
roll-by-3 (a pure layout permutation).

Layout: partition p holds rows 32p..32p+31 as 32 streams of width TW=1256
in the free dim.  Row shifts i->i+1 are stream shifts (intra-partition)
except stream 31 -> next partition's stream 0, handled by two small
partition-shift SBUF->SBUF DMAs (A = S_0[p+1], U' = U_{0,1}[p+1]) whose
boundary row 127 is memset to zero -- which also zeroes the 3 invalid
n-gram starts 4093..4095 automatically.

Pipeline: 8 groups of 4 streams. Per group: feat DMA + signal gather ->
bind (S = sig*feat, in place over feat) -> U_s = S_s . S_{s+1}(+1) ->
Q_s = U_s . U_{s+2}(+2) -> T_t = Q_t + Q_{t+16} -> PSUM-accumulated
ones-matmul over partitions (3 segs x 16 streams).  U overwrites the dead
gathered-signal buffer, Q overwrites dead S, T overwrites dead U, so peak
SBUF is ~2 x 80KB/partition.

Index quantization is bit-exact via a host-built bucket LUT: b =
trunc/round((x-MIN)*NB/RANGE) (any rounding within +-1 bucket is fine by
construction), then idx = base[b] + (x >= t[b]) where each bucket's
extended window provably contains at most one of the 999 exact fp32
thresholds (bisected so that #{thr <= x} == reference idx).

All values are +/-1 so bf16 is exact; bundle partials are integers < 2^12
so fp32 PSUM is exact; the output sign never sees zero (4093 odd terms).
"""
import sys

sys.path.insert(0, "/opt/trn_rl_repo")

import numpy as np
import ml_dtypes

import concourse.bass as bass
import concourse.bacc as bacc
import concourse.tile as tile
import concourse.mybir as mybir
from concourse.bass_utils import run_bass_kernel_spmd
from concourse import library_config

# ---- problem constants ----
MAX_VAL = 52000.0
MIN_VAL = -53000.0
RANGE = MAX_VAL - MIN_VAL
NUM_LEVELS = 1000
NGRAM = 4
D = 10000
NFEAT = 4096
NCORE = 8

ROLL = NGRAM - 1
SLICE = D // NCORE            # 1250 output cols per core

NS = 32                       # streams (rows) per partition
TW = 1280                     # stream pitch (2560B, dma_gather 256B-multiple)
NG = 8                        # pipeline groups
GS = NS // NG                 # 4 streams per group

NB = 4096                     # quantizer buckets
NBP = NB + 4                  # padded bucket table rows
BSCALE = float(np.float32(NB / RANGE))

SEGS = [(0, 512), (512, 1024), (1024, 1252)]   # PSUM-bank matmul segs

F32 = mybir.dt.float32
BF16 = mybir.dt.bfloat16
I32 = mybir.dt.int32
I16 = mybir.dt.int16
_BF = ml_dtypes.bfloat16

NTH = NUM_LEVELS - 1


# ---------------------------------------------------------------- host prep
def _f2o(u):
    b = u.view(np.uint32).astype(np.int64)
    return np.where(b < 0x80000000, b + 0x80000000, 0xFFFFFFFF - b)


def _o2f(o):
    b = np.where(o >= 0x80000000, o - 0x80000000, 0xFFFFFFFF - o).astype(np.uint64)
    return b.astype(np.uint32).view(np.float32)


def _g(v):
    v = v.astype(np.float32)
    t = (v - np.float32(MIN_VAL)).astype(np.float32)
    t = (t / np.float32(MAX_VAL - MIN_VAL)).astype(np.float32)
    t = (t * np.float32(NUM_LEVELS - 1)).astype(np.float32)
    return np.clip(np.round(t), 0.0, float(NUM_LEVELS - 1))


def _thresholds():
    ks = np.arange(1, NUM_LEVELS, dtype=np.float32)
    lo = _f2o(np.full(ks.shape, np.float32(MIN_VAL) - np.float32(2.0)))
    hi = _f2o(np.full(ks.shape, np.float32(MAX_VAL) + np.float32(2.0)))
    for _ in range(64):
        mid = (lo + hi) // 2
        ge = _g(_o2f(mid)) >= ks
        hi = np.where(ge, mid, hi)
        lo = np.where(ge, lo, mid)
        if np.all(hi - lo <= 1):
            break
    return _o2f(hi)


def _bucket_table():
    """(NBP,) t and (NBP,) base f32 tables: idx(x) = base[b] + (x >= t[b])
    for any device bucket b within +-1.02 of (x-MIN)*NB/RANGE."""
    thr = _thresholds().astype(np.float64)          # (999,) sorted
    w = RANGE / NB
    t = np.full(NBP, 3.0e38, dtype=np.float32)
    base = np.zeros(NBP, dtype=np.float32)
    bs = np.arange(NBP, dtype=np.float64)
    lo = MIN_VAL + (bs - 1.02) * w
    hi = MIN_VAL + (bs + 1.02) * w
    for b in range(NBP):
        inb = np.nonzero((thr > lo[b]) & (thr <= hi[b]))[0]
        assert len(inb) <= 1, f"bucket {b} holds {len(inb)} thresholds"
        base[b] = np.count_nonzero(thr <= lo[b])
        if len(inb):
            t[b] = np.float32(thr[inb[0]])
    w64 = np.zeros((NBP, 64), dtype=np.float32)
    w64[:, 0] = t
    w64[:, 1] = base
    return w64


_CACHE = {}


def _host_constants():
    if "thr" not in _CACHE:
        _CACHE["thr"] = np.tile(_thresholds()[None, :], (128, 1)).astype(np.float32)
        _CACHE["zrow"] = np.zeros((1, 2 * TW), dtype=_BF)
    return _CACHE


# ---------------------------------------------------------------- program
def _build_program():
    nc = bacc.Bacc("TRN2", target_bir_lowering=False, debug=False,
                   num_devices=NCORE)

    x32_d = nc.dram_tensor("x32", (128, NS), F32, kind="ExternalInput")
    thr_d = nc.dram_tensor("thr", (128, NTH), F32, kind="ExternalInput")
    table_d = nc.dram_tensor("table", (NUM_LEVELS, TW), BF16,
                             kind="ExternalInput")
    feat_d = nc.dram_tensor("feat", (NG, 128, GS * TW), BF16,
                            kind="ExternalInput")
    xsl_d = nc.dram_tensor("xsl", (128, 4), F32, kind="ExternalInput")
    zrow_d = nc.dram_tensor("zrow", (1, 2 * TW), BF16, kind="ExternalInput")
    cc_in8 = nc.dram_tensor("cc_in8", (NCORE, 512), F32)
    cc_out = nc.dram_tensor("cc_out", (NCORE, 512), F32)
    out_d = nc.dram_tensor("out", (1, SLICE), F32, kind="ExternalOutput")
    if DEBUG:
        dbg_idx_d = nc.dram_tensor("dbg_idx", (128, NS), I32,
                                   kind="ExternalOutput")
        dbg_sig_d = nc.dram_tensor("dbg_sig", (128, TW), BF16,
                                   kind="ExternalOutput")
        dbg_s_d = nc.dram_tensor("dbg_s", (128, TW), BF16,
                                 kind="ExternalOutput")
        dbg_acc_d = nc.dram_tensor("dbg_acc", (1, 1252), F32,
                                   kind="ExternalOutput")

    # raw tensors for partition-shifted copies (row 127 stays zero)
    a_raw = nc.alloc_sbuf_tensor("a_shift", [128, TW], BF16).ap()
    up_raw = nc.alloc_sbuf_tensor("up_shift", [128, 2 * TW], BF16).ap()

    with tile.TileContext(nc) as tc:
        with tc.tile_pool(name="const", bufs=1) as cpool, \
             tc.tile_pool(name="work", bufs=1) as wpool, \
             tc.tile_pool(name="pacc", bufs=1, space="PSUM") as pacc:

            # ---- constants / index computation ----
            onr = cpool.tile([128, 1], BF16, tag="onr")
            nc.vector.memset(onr[:, :], 1.0)
            nc.scalar.dma_start(out=a_raw[127:128, :], in_=zrow_d[0:1, 0:TW])
            nc.scalar.dma_start(out=up_raw[127:128, :], in_=zrow_d[0:1, :])

            x32 = cpool.tile([128, NS], F32, tag="x32")
            nc.sync.dma_start(out=x32[:, :], in_=x32_d[:, :])
            thr = cpool.tile([128, NTH], F32, tag="thr")
            nc.sync.dma_start(out=thr[:, :], in_=thr_d[:, :])

            # idx[p, s] = #{thr <= x[p, s]} via is_le compare with fused
            # free-dim accumulate.  Streams 0..7 computed locally (fast
            # start); each core computes its 4-stream slice (xsl) and an
            # AllToAll distributes the full idx table off the DVE.
            ge = cpool.tile([128, NTH], BF16, tag="ge")
            idxf = cpool.tile([128, NS], F32, tag="idxf")
            idxn = cpool.tile([128, NS], I32, tag="idxn")
            idxfl = cpool.tile([128, 8], F32, tag="idxfl")
            idxnl = cpool.tile([128, 8], I32, tag="idxnl")
            xsl = cpool.tile([128, 4], F32, tag="xsl")
            nc.sync.dma_start(out=xsl[:, :], in_=xsl_d[:, :])
            idxsl = cpool.tile([128, 4], F32, tag="idxsl")

            for j in range(4):
                nc.vector.tensor_scalar(
                    out=ge[:, :], in0=thr[:, :],
                    scalar1=xsl[:, j:j + 1], scalar2=0.0,
                    op0=mybir.AluOpType.is_le,
                    op1=mybir.AluOpType.add,
                    accum_out=idxsl[:, j:j + 1])
            for r in range(NCORE):
                nc.sync.dma_start(out=cc_in8[r:r + 1, :], in_=idxsl[:, :])
            nc.gpsimd.collective_compute(
                "AllToAll", mybir.AluOpType.bypass,
                ins=[cc_in8[:, :]], outs=[cc_out[:, :]],
                replica_groups=[list(range(NCORE))])
            idxf_3 = idxf[:, :].rearrange("p (m j) -> p m j", m=NCORE)
            nc.sync.dma_start(
                out=idxf_3[:, :, :],
                in_=cc_out[:, :].rearrange("m (p j) -> p m j", p=128))
            nc.vector.tensor_copy(out=idxn[:, :], in_=idxf[:, :])

            # local streams 0..7 (pace the first two gather groups)
            for s in range(8):
                nc.vector.tensor_scalar(
                    out=ge[:, :], in0=thr[:, :],
                    scalar1=x32[:, s:s + 1], scalar2=0.0,
                    op0=mybir.AluOpType.is_le,
                    op1=mybir.AluOpType.add,
                    accum_out=idxfl[:, s:s + 1])
            nc.vector.tensor_copy(out=idxnl[:, :], in_=idxfl[:, :])
            if DEBUG:
                nc.sync.dma_start(out=dbg_idx_d[:, :], in_=idxn[:, :])

            # ---- main buffers ----
            sb = wpool.tile([128, NS * TW], BF16, tag="sb")    # feat -> S -> Q
            gb = wpool.tile([128, NS * TW], BF16, tag="gb")    # sig -> U -> T
            sb_r = sb[:, :].rearrange("p (s w) -> p s w", s=NS)
            gb_r = gb[:, :].rearrange("p (s w) -> p s w", s=NS)

            acc = pacc.tile([1, 1252], F32, tag="acc")

            def u_window(lo, hi):
                """U_s = S_s * S_{s+1}[+1] for s in [lo, hi) (intra-partition)."""
                nc.vector.tensor_tensor(
                    out=gb_r[:, lo:hi, 0:1254],
                    in0=sb_r[:, lo:hi, 0:1254],
                    in1=sb_r[:, lo + 1:hi + 1, 1:1255],
                    op=mybir.AluOpType.mult)

            def q_window(lo, hi):
                """Q_s = U_s * U_{s+2}[+2] for s in [lo, hi) (intra-partition)."""
                nc.vector.tensor_tensor(
                    out=sb_r[:, lo:hi, 0:1252],
                    in0=gb_r[:, lo:hi, 0:1252],
                    in1=gb_r[:, lo + 2:hi + 2, 2:1254],
                    op=mybir.AluOpType.mult)

            def q_matmuls(s):
                """accumulate Q_s (in sb) into the PSUM bundle accumulator."""
                for a0, a1 in SEGS:
                    nc.tensor.matmul(out=acc[0:1, a0:a1],
                                     lhsT=onr[:, 0:1],
                                     rhs=sb[:, s * TW + a0:s * TW + a1],
                                     start=(s == 0), stop=(s == 31))

            # ---- pipelined groups ----
            for g in range(NG):
                s0 = g * GS
                nc.sync.dma_start(out=sb[:, s0 * TW:(s0 + GS) * TW],
                                  in_=feat_d[g, :, :])
                for j in range(GS):
                    s = g * GS + j
                    it = idxnl[:, s:s + 1] if g < 2 else idxn[:, s:s + 1]
                    nc.gpsimd.indirect_dma_start(
                        out=gb[:, s * TW:(s + 1) * TW], out_offset=None,
                        in_=table_d[:, :],
                        in_offset=bass.IndirectOffsetOnAxis(ap=it, axis=0),
                        element_offset=0)
                if DEBUG and g == 0:
                    nc.sync.dma_start(out=dbg_sig_d[:, :], in_=gb[:, 0:TW])
                # bind S = sig * feat (in place over feat)
                nc.vector.tensor_tensor(
                    out=sb_r[:, s0:s0 + GS, :],
                    in0=sb_r[:, s0:s0 + GS, :],
                    in1=gb_r[:, s0:s0 + GS, :],
                    op=mybir.AluOpType.mult)
                if DEBUG and g == 0:
                    nc.sync.dma_start(out=dbg_s_d[:, :], in_=sb[:, 0:TW])

                if g == 0:
                    # A[p] = S_0[p+1] for U_31 (boundary row 127 is zero)
                    nc.scalar.dma_start(out=a_raw[0:127, :],
                                        in_=sb[1:128, 0:TW])
                    u_window(0, GS - 1)                      # U_0..2
                    # U'[p] = U_{0,1}[p+1] for Q_30,31
                    nc.scalar.dma_start(out=up_raw[0:127, :],
                                        in_=gb[1:128, 0:2 * TW])
                else:
                    u_window(s0 - 1, s0 + GS - 1)            # U_{4g-1}..{4g+2}
                    # Q streams [4(g-1) .. 4(g-1)+3] need U <= 4g+1 (done)
                    q0 = (g - 1) * GS
                    q_window(q0, q0 + GS)
                    for s in range(q0, q0 + GS):
                        q_matmuls(s)

            # ---- tail: boundary streams ----
            # U_31 = S_31 * A[+1]  (all 2D APs)
            nc.vector.tensor_tensor(
                out=gb[:, 31 * TW:31 * TW + 1254],
                in0=sb[:, 31 * TW:31 * TW + 1254],
                in1=a_raw[:, 1:1255],
                op=mybir.AluOpType.mult)
            # Q_28,29 (need U_30, U_31)
            q_window(28, 30)
            # Q_30 = U_30 * U'_0[+2];  Q_31 = U_31 * U'_1[+2]
            up_r = up_raw[:, :].rearrange("p (s w) -> p s w", s=2)
            nc.vector.tensor_tensor(
                out=sb_r[:, 30:32, 0:1252],
                in0=gb_r[:, 30:32, 0:1252],
                in1=up_r[:, 0:2, 2:1254],
                op=mybir.AluOpType.mult)
            for s in range(28, 32):
                q_matmuls(s)

            # ---- sign + output ----
            if DEBUG:
                dacc = wpool.tile([1, 1252], F32, tag="dacc")
                nc.scalar.copy(out=dacc[:, :], in_=acc[0:1, :])
                nc.sync.dma_start(out=dbg_acc_d[0:1, :], in_=dacc[:, :])
            t1 = wpool.tile([1, SLICE], F32, tag="fin2")
            nc.vector.tensor_scalar(out=t1[:, :], in0=acc[0:1, 0:SLICE],
                                    scalar1=0.0, scalar2=2.0,
                                    op0=mybir.AluOpType.is_gt,
                                    op1=mybir.AluOpType.mult)
            sg = wpool.tile([1, SLICE], F32, tag="fin3")
            nc.vector.tensor_scalar(out=sg[:, :], in0=t1[:, :], scalar1=-1.0,
                                    scalar2=None, op0=mybir.AluOpType.add)
            nc.sync.dma_start(out=out_d[0:1, :], in_=sg[:, :])

    nc.compile()
    return nc


TRACE = False
DEBUG = False
LAST_RESULT = None


def _make_in_maps(xf, sw, fw, consts):
    in_maps = []
    x32 = xf.reshape(128, NS).astype(np.float32)

    for m in range(NCORE):
        c0 = SLICE * m
        cols = (c0 + np.arange(TW)) % D
        table = sw[:, cols].astype(_BF)                       # (1000, TW)
        fwc = fw[:, cols].astype(_BF)                         # (4096, TW)
        feat = np.ascontiguousarray(
            fwc.reshape(128, NG, GS, TW)
               .transpose(1, 0, 2, 3)
               .reshape(NG, 128, GS * TW))
        xsl = np.ascontiguousarray(
            xf.reshape(128, NS)[:, 4 * m:4 * m + 4]).astype(np.float32)
        in_maps.append({
            "x32": x32,
            "xsl": xsl,
            "thr": consts["thr"],
            "table": table,
            "feat": feat,
            "zrow": consts["zrow"],
        })
    return in_maps


def kernel(x, signals_weight, feat_weight):
    global LAST_RESULT
    consts = _host_constants()

    if "nc" not in _CACHE:
        _CACHE["nc"] = _build_program()
    nc = _CACHE["nc"]

    xf = np.asarray(x, dtype=np.float32).reshape(-1)
    sw = np.asarray(signals_weight, dtype=np.float32)
    fw = np.asarray(feat_weight, dtype=np.float32)
    in_maps = _make_in_maps(xf, sw, fw, consts)

    res = run_bass_kernel_spmd(nc, in_maps, list(range(NCORE)), trace=TRACE)
    LAST_RESULT = res
    full = np.concatenate(
        [np.asarray(res.results[m]["out"], dtype=np.float32).reshape(-1)
         for m in range(NCORE)])
    return np.roll(full, ROLL)[None, :]
